# revision 2
# baseline (speedup 1.0000x reference)
"""Trainium2 Bass kernel v3 for nn_ActionHead_46411416600827.

Host precomputes U = embeds @ W1[:128] and V = embeds @ W1[128:256] (f32),
reducing the device per-action work to logits = relu(U[atk] + V[dfd] + ws*n
+ b1) @ w2 + b2. U/V rows are packed in pairs ([row2i, row2i+1] = 512B bf16
elements) and fetched with one normal-mode dma_gather per action per table:
half-indices fit int16 (<= 25001) and 512B descriptors run the DMA bus at
full rate (2x the 256B baseline). Actions are host-sorted into 4 parity
classes (atk&1, dfd&1), so selecting the wanted half of each pair is a
static slice. Per 128-action slot: DVE computes xz = ws*n + xa + xd in
action-major (n is a per-partition scalar there), PE transposes xz to
embedding-major bf16 PSUM, ScalarE applies relu(+b1), PE contracts with w2,
DVE adds b2. Skip actions (atk == -1, logit depends only on n) and any
class-region overflow are computed exactly on the host from the f32 tables.
"""
import sys

sys.path.insert(0, "/opt/trn_rl_repo")
import numpy as np
import ml_dtypes
import concourse.bass as bass
import concourse.bacc as bacc
import concourse.mybir as mybir
import concourse.tile as tile
from concourse.masks import make_identity
from concourse import bass_utils

P = 128
D = 128
HID = 128
N_NODES = 50000
NUM_ACTIONS = 1_000_000
N_CORES = 8
PER_CORE = NUM_ACTIONS // N_CORES  # 125000

f32 = mybir.dt.float32
bf16 = mybir.dt.bfloat16
i16 = mybir.dt.int16

# +1-shifted row space: 0 zero | 1..50000 nodes | 50001 skip_atk | 50002 skip_dfd | 50003 pad
SKIP_ATK_ROW = N_NODES + 1
SKIP_DFD_ROW = N_NODES + 2
TABLE_ROWS = N_NODES + 4     # 50004 (even)
THALF = TABLE_ROWS // 2      # 25002 packed-pair elements

REG = 32768                  # padded actions per parity class per core
TOT = 4 * REG                # 131072 padded actions per core
CHUNK = 1024                 # actions per dma_gather call (ring capacity)
NS = CHUNK // P              # 8 slots per chunk
NCHUNK = TOT // CHUNK        # 128
CPCLS = REG // CHUNK         # 32 chunks per class


def build_kernel(n_actions_core=TOT, nqueues=1, gbufs=4):
    assert n_actions_core == TOT
    ncol = CHUNK // 16  # idx columns per chunk in wrapped layout

    kw = {}
    if nqueues > 1:
        kw["num_swdge_queues"] = nqueues
    nc = bacc.Bacc("TRN2", num_devices=N_CORES, debug=False,
                   target_bir_lowering=False, dynamic_dma_scratch_size=65536,
                   **kw)

    tu_d = nc.dram_tensor("tu", [THALF, 2 * D], bf16, kind="ExternalInput")
    tv_d = nc.dram_tensor("tv", [THALF, 2 * D], bf16, kind="ExternalInput")
    wa_d = nc.dram_tensor("wa", [32, TOT // 16], i16, kind="ExternalInput")
    wd_d = nc.dram_tensor("wd", [32, TOT // 16], i16, kind="ExternalInput")
    nr_d = nc.dram_tensor("nr", [NCHUNK, P, NS], bf16, kind="ExternalInput")
    wsr_d = nc.dram_tensor("wsr", [P, HID], bf16, kind="ExternalInput")
    w2_d = nc.dram_tensor("w2", [HID, 1], bf16, kind="ExternalInput")
    b1_d = nc.dram_tensor("b1", [HID], f32, kind="ExternalInput")
    b2r_d = nc.dram_tensor("b2r", [P], f32, kind="ExternalInput")
    out_d = nc.dram_tensor("logits_dev", [NCHUNK, 2, 512], f32,
                           kind="ExternalOutput")

    relu = mybir.ActivationFunctionType.Relu

    with tile.TileContext(nc) as tc:
        with (
            tc.tile_pool(name="const", bufs=1) as cb,
            tc.tile_pool(name="sb_g", bufs=gbufs) as sb_g,
            tc.tile_pool(name="sb_n", bufs=3) as sb_n,
            tc.tile_pool(name="sb_x", bufs=3) as sb_x,
            tc.tile_pool(name="sb_h", bufs=3) as sb_h,
            tc.tile_pool(name="sb_l", bufs=2) as sb_l,
            tc.tile_pool(name="ps_t", bufs=2, space="PSUM") as ps_t,
            tc.tile_pool(name="ps_l", bufs=2, space="PSUM") as ps_l,
        ):
            ident = cb.tile([P, P], bf16)
            make_identity(nc, ident[:])
            wsrep = cb.tile([P, HID], bf16)
            nc.sync.dma_start(out=wsrep[:], in_=wsr_d.ap())
            w2c = cb.tile([HID, 1], bf16)
            nc.sync.dma_start(out=w2c[:], in_=w2_d.ap())
            b1c = cb.tile([HID, 1], f32)
            nc.sync.dma_start(out=b1c[:], in_=b1_d.ap()[:, None])
            b2c = cb.tile([P, 1], f32)
            nc.sync.dma_start(out=b2c[:], in_=b2r_d.ap()[:, None])
            wa_sb = cb.tile([32, TOT // 16], i16)
            nc.sync.dma_start(out=wa_sb[:], in_=wa_d.ap())
            wd_sb = cb.tile([32, TOT // 16], i16)
            nc.sync.dma_start(out=wd_sb[:], in_=wd_d.ap())

            for c in range(NCHUNK):
                pa = (c // CPCLS) >> 1
                pd = (c // CPCLS) & 1
                ga = sb_g.tile([P, NS * 2 * D], bf16, tag="ga")
                gd = sb_g.tile([P, NS * 2 * D], bf16, tag="gd")
                nc.gpsimd.dma_gather(
                    out_ap=ga[:].rearrange("p (s d) -> p s d", s=NS),
                    in_ap=tu_d.ap(),
                    idxs_ap=wa_sb[:, c * ncol : (c + 1) * ncol],
                    num_idxs=CHUNK, num_idxs_reg=CHUNK, elem_size=2 * D,
                    queue_num=0,
                )
                nc.gpsimd.dma_gather(
                    out_ap=gd[:].rearrange("p (s d) -> p s d", s=NS),
                    in_ap=tv_d.ap(),
                    idxs_ap=wd_sb[:, c * ncol : (c + 1) * ncol],
                    num_idxs=CHUNK, num_idxs_reg=CHUNK, elem_size=2 * D,
                    queue_num=(1 if nqueues > 1 else 0),
                )
                nt = sb_n.tile([P, NS], bf16, tag="nt")
                nc.sync.dma_start(out=nt[:], in_=nr_d.ap()[c])

                xz = sb_x.tile([P, NS * D], bf16, tag="xz")
                zpt = ps_t.tile([P, NS * D], bf16, tag="zpt")
                for s in range(NS):
                    xsl = slice(s * D, (s + 1) * D)
                    asl = slice(s * 2 * D + pa * D, s * 2 * D + (pa + 1) * D)
                    dsl = slice(s * 2 * D + pd * D, s * 2 * D + (pd + 1) * D)
                    nc.vector.scalar_tensor_tensor(
                        out=xz[:, xsl], in0=wsrep[:], scalar=nt[:, s : s + 1],
                        in1=ga[:, asl], op0=mybir.AluOpType.mult,
                        op1=mybir.AluOpType.add)
                    nc.vector.tensor_tensor(
                        out=xz[:, xsl], in0=xz[:, xsl], in1=gd[:, dsl],
                        op=mybir.AluOpType.add)
                    nc.tensor.transpose(
                        out=zpt[:, xsl], in_=xz[:, xsl], identity=ident[:])

                h = sb_h.tile([P, NS * D], bf16, tag="h")
                nc.scalar.activation(out=h[:], in_=zpt[:], func=relu,
                                     bias=b1c[:])
                lps = ps_l.tile([P, 512], f32, tag="lps")
                for q2 in range(2):
                    nc.tensor.matmul(
                        out=lps[64 * q2 : 64 * q2 + 1, :], lhsT=w2c[:],
                        rhs=h[:, q2 * 512 : (q2 + 1) * 512],
                        start=True, stop=True)
                lsb = sb_l.tile([P, 512], f32, tag="lsb")
                nc.vector.tensor_scalar(
                    out=lsb[:], in0=lps[:], scalar1=b2c[:], scalar2=None,
                    op0=mybir.AluOpType.add)
                nc.sync.dma_start(out=out_d.ap()[c], in_=lsb[::64, :])

    nc.compile()
    return nc


def host_prep(inputs, sort_by_idx=True):
    node = np.asarray(inputs["node_embeddings"], dtype=np.float32)
    ska = np.asarray(inputs["skip_attack_embed"], dtype=np.float32)
    skd = np.asarray(inputs["skip_defend_embed"], dtype=np.float32)
    w1 = np.asarray(inputs["W1"], dtype=np.float32)
    b1 = np.asarray(inputs["b1"], dtype=np.float32)
    w2 = np.asarray(inputs["W2"], dtype=np.float32)
    b2 = np.asarray(inputs["b2"], dtype=np.float32)

    w1a, w1d, wsv = w1[:D], w1[D : 2 * D], w1[2 * D]
    U = np.zeros((TABLE_ROWS, D), dtype=np.float32)
    V = np.zeros((TABLE_ROWS, D), dtype=np.float32)
    U[1 : N_NODES + 1] = node @ w1a
    V[1 : N_NODES + 1] = node @ w1d
    U[SKIP_ATK_ROW] = ska @ w1a
    V[SKIP_DFD_ROW] = skd @ w1d
    tu = U.astype(ml_dtypes.bfloat16).reshape(THALF, 2 * D)
    tv = V.astype(ml_dtypes.bfloat16).reshape(THALF, 2 * D)

    alt = np.asarray(inputs["action_lookup_table"])
    A = alt.shape[0]
    assert A == NUM_ACTIONS

    wsrep = np.tile(wsv.astype(ml_dtypes.bfloat16)[None, :], (P, 1))
    w2b = w2.astype(ml_dtypes.bfloat16).reshape(HID, 1)
    b2r = np.repeat(b2, P).astype(np.float32)

    in_maps = []
    metas = []
    for core in range(N_CORES):
        lo = core * PER_CORE
        sh = alt[lo : lo + PER_CORE]
        atk = sh[:, 0].astype(np.int64)
        dfd = sh[:, 1].astype(np.int64)
        nso = sh[:, 2].astype(np.float32)
        skip = atk < 0
        ia = np.where(skip, SKIP_ATK_ROW, atk + 1).astype(np.int32)
        idd = np.where(skip, SKIP_DFD_ROW, dfd + 1).astype(np.int32)
        cls = (ia & 1) * 2 + (idd & 1)

        nonskip = np.nonzero(~skip)[0]
        if sort_by_idx:
            # secondary sort by atk half-index: U-table gather descriptors
            # walk ascending HBM addresses (DRAM row-buffer friendly)
            order = nonskip[np.lexsort((ia[nonskip] >> 1, cls[nonskip]))]
        else:
            order = nonskip[np.argsort(cls[nonskip], kind="stable")]
        counts = np.bincount(cls[order], minlength=4)

        host_idx = list(np.nonzero(skip)[0])

        ia_p = np.zeros(TOT, dtype=np.int32)
        id_p = np.zeros(TOT, dtype=np.int32)
        n_p = np.zeros(TOT, dtype=np.float32)
        src = np.full(TOT, -1, dtype=np.int64)
        soff = 0
        for cl in range(4):
            cnt = int(counts[cl])
            seg = order[soff : soff + cnt]
            if cnt > REG:
                host_idx.extend(seg[REG:])
                seg = seg[:REG]
                cnt = REG
            off = cl * REG
            ia_p[off : off + cnt] = ia[seg]
            id_p[off : off + cnt] = idd[seg]
            n_p[off : off + cnt] = nso[seg]
            src[off : off + cnt] = seg
            ia_p[off + cnt : off + REG] = cl >> 1
            id_p[off + cnt : off + REG] = cl & 1
            soff += int(counts[cl])

        hi = np.asarray(host_idx, dtype=np.int64)
        if hi.size:
            zh = (
                U[ia[hi]] + V[idd[hi]]
                + nso[hi][:, None] * wsv[None, :]
                + b1[None, :]
            )
            host_lg = np.maximum(zh, 0.0) @ w2[:, 0] + b2[0]
        else:
            host_lg = np.zeros(0, dtype=np.float32)

        def wrap(idx_half):
            w = idx_half.reshape(NCHUNK, CHUNK // 16, 16).transpose(2, 0, 1)
            w = w.reshape(16, NCHUNK * (CHUNK // 16))
            return np.ascontiguousarray(np.concatenate([w, w], axis=0))

        m = {
            "tu": tu,
            "tv": tv,
            "wa": wrap((ia_p >> 1).astype(np.int16)),
            "wd": wrap((id_p >> 1).astype(np.int16)),
            # action-major n: nr[c, p, s] = n of action c*CHUNK + s*128 + p
            "nr": np.ascontiguousarray(
                n_p.astype(ml_dtypes.bfloat16)
                .reshape(NCHUNK, NS, P)
                .transpose(0, 2, 1)
            ),
            "wsr": wsrep,
            "w2": w2b,
            "b1": b1,
            "b2r": b2r,
        }
        in_maps.append(m)
        metas.append({"src": src, "hi": hi, "host_lg": host_lg})
    return in_maps, {"A": A, "metas": metas}


def host_post(results, meta):
    out = np.empty(NUM_ACTIONS, dtype=np.float32)
    for core in range(N_CORES):
        lg = results[core]["logits_dev"]  # [NCHUNK, 2, 512]
        x = lg.reshape(TOT)  # action = c*1024 + q2*512 + col
        mc = meta["metas"][core]
        src = mc["src"]
        valid = src >= 0
        lo = core * PER_CORE
        seg = out[lo : lo + PER_CORE]
        seg[src[valid]] = x[valid]
        if mc["hi"].size:
            seg[mc["hi"]] = mc["host_lg"]
    return out


def run_full(inputs, trace=False):
    in_maps, meta = host_prep(inputs)
    nc = build_kernel()
    res = bass_utils.run_bass_kernel_spmd(
        nc, in_maps, core_ids=list(range(N_CORES)), trace=trace
    )
    return host_post(res.results, meta), res


def kernel(**inputs):
    out, _res = run_full(inputs)
    return out
